# revision 18
# baseline (speedup 1.0000x reference)
"""Trainium2 Bass kernel for nn_Projection: out = [(1-s)*x, s],
s = -(1-||x||^2)/(1+||x||^2) per row.

Identity used: with sq = sum(x^2), s = (sq-1)/(sq+1) = 1 - 2/(1+sq).
Let t = 2/(1+sq). Then out = [t*x, 1-t].

HBM-bandwidth bound (elementwise over 512MB in / 516MB out). Gate is
rel_err < 2e-2, so all HBM traffic is bf16 (measured max rel err
~1.2e-2): the host rounds x to bf16, the device computes and stores
bf16, the host upcasts to f32. Halves HBM traffic vs f32.

Layout trick: tiles are d-major in SBUF ([P, D, blk], host pre/post
transposes the per-tile element order). This makes every hot DVE op
eligible for the 2-byte packed 2x perf mode (innermost step 1 on all
operands, including the per-row t broadcast which is stride-0 only
in the middle dim):
  - row-sum of x^2: 7-level fp16 fold tree over d, done IN-PLACE in
    the xsq tile (the write pointer trails both read pointers)
  - t*x multiply: one 2x TT with t16 broadcast over d
GpSimd stays idle: any GpSimd op would serialize with DVE 2x ops on
the exclusive shared SBUF port pair.

Software-pipelined with a 2-chunk skew so the in-order ACT/DVE
queues never stall on each other; loads/stores alternate between the
two HWDGE rings so single-direction phases can use both. The chunk
schedule is variable-size: small chunks at the ends shorten pipeline
fill/drain (dep-chain length scales with chunk size), 64-row chunks
in the middle amortize per-instruction overhead.

Sharding: pure data parallel over rows across 8 NeuronCores.
Per-core row map: row = p*K + (chunk offset + j).
"""

import sys

for _p in ("/opt/trn_rl_repo", "/opt/trn_rl_repo/concourse"):
    if _p not in sys.path:
        sys.path.insert(0, _p)

import ml_dtypes
import numpy as np

import concourse.bacc as bacc
import concourse.tile as tile
from concourse import mybir
from concourse.bass_utils import run_bass_kernel_spmd

N, D = 1048576, 128
N_CORES = 8
R = N // N_CORES   # 131072 rows per core
P = 128            # SBUF partitions
K = R // P         # 1024 rows per partition
BMAX = 64
CHUNKS = [16, 48] + [64] * 14 + [32, 32]      # rows per partition per chunk
assert sum(CHUNKS) == K
OFFS = [sum(CHUNKS[:i]) for i in range(len(CHUNKS))]
BF16 = mybir.dt.bfloat16
FP16 = mybir.dt.float16
F32 = mybir.dt.float32
NP_BF16 = np.dtype(ml_dtypes.bfloat16)


def build_nc(pre: int = 3):
    nch = len(CHUNKS)
    nc = bacc.Bacc(trn_type="TRN2")
    x = nc.dram_tensor("x", [P * K * D], BF16, kind="ExternalInput")
    tx = nc.dram_tensor("tx", [P * K * D], BF16, kind="ExternalOutput")
    s = nc.dram_tensor("s", [P, K], BF16, kind="ExternalOutput")

    def dview(t, c):
        base = OFFS[c] * P * D
        return t.ap()[base:base + P * D * CHUNKS[c]].rearrange(
            "(p f) -> p f", p=P)

    with tile.TileContext(nc) as tc:
        with (
            tc.tile_pool(name="xin", bufs=pre + 6) as x_pool,
            tc.tile_pool(name="sqp", bufs=2) as sq_pool,
            tc.tile_pool(name="small", bufs=4) as small_pool,
            tc.tile_pool(name="singles", bufs=1) as singles,
        ):
            s_all = singles.tile([P, K], BF16)

            x_t = {}     # c -> x tile view [P, D, blk]
            x_fl = {}    # c -> flat x tile
            xsq = {}     # c -> squared tile view
            sqs = {}     # c -> row-sum (fp16 [P, blk])
            us = {}      # c -> u = sq + 0.5 (f32)
            t32s = {}    # c -> 1/u (f32)

            def view(flat, blk):
                return flat[:, 0:D * blk].rearrange("p (d j) -> p d j", j=blk)

            def load(c):
                # Half-transfers on BOTH rings: per-ring throughput caps at
                # ~215-225 GB/s; both together sustain 430+ (direction-
                # agnostic), so every transfer is split across the two.
                blk = CHUNKS[c]
                fl = x_pool.tile([P, D * BMAX + BMAX], BF16, tag="x", name="x_t")
                half = D * blk // 2
                dv = dview(x, c)
                nc.scalar.dma_start(out=fl[:, 0:half], in_=dv[:, 0:half])
                nc.sync.dma_start(out=fl[:, half:2 * half],
                                  in_=dv[:, half:2 * half])
                x_t[c] = view(fl, blk)
                x_fl[c] = fl

            def square(c):
                fl = sq_pool.tile([P, D * BMAX], FP16, tag="xsq", name="xsq")
                xsq[c] = view(fl, CHUNKS[c])
                nc.scalar.activation(
                    out=xsq[c], in_=x_t[c],
                    func=mybir.ActivationFunctionType.Square,
                    scale=0.7071067811865476,
                )

            def folds(c):
                # In-place fold tree over d: write trails both reads.
                blk = CHUNKS[c]
                a = xsq[c]
                d = D
                while d > 2:
                    d //= 2
                    nc.vector.tensor_add(
                        a[:, 0:d, :], a[:, 0:d, :], a[:, d:2 * d, :])
                sq_f = small_pool.tile([P, BMAX], FP16, tag="sq", name="sq")
                sqs[c] = sq_f[:, 0:blk]
                nc.vector.tensor_add(
                    sqs[c].unsqueeze(1), a[:, 0:1, :], a[:, 1:2, :])
                del xsq[c]

            def u_of(c):
                u_f = small_pool.tile([P, BMAX], F32, tag="u", name="u")
                us[c] = u_f[:, 0:CHUNKS[c]]
                nc.scalar.activation(
                    out=us[c], in_=sqs[c],
                    func=mybir.ActivationFunctionType.Copy, bias=0.5,
                )
                del sqs[c]

            def tail(c):
                blk = CHUNKS[c]
                t_f = small_pool.tile([P, BMAX], F32, tag="t32", name="t32")
                t32s[c] = t_f[:, 0:blk]
                nc.vector.reciprocal_approx_fast(out=t32s[c], in_=us[c])
                del us[c]
                fl = x_fl[c]
                # t16 lives INSIDE the x tile (spare tail columns): the 4x
                # DVE mode (which the in-place folds hit) engages when out
                # aliases in0 AND both sources come from the same tensor.
                t16 = fl[:, D * blk:D * blk + blk]
                nc.vector.tensor_copy(t16, t32s[c])
                nc.vector.tensor_mul(
                    x_t[c], x_t[c],
                    t16.unsqueeze(1).broadcast_to([P, D, blk]))
                del x_t[c], x_fl[c]
                half = D * blk // 2
                dv = dview(tx, c)
                nc.sync.dma_start(out=dv[:, 0:half], in_=fl[:, 0:half])
                nc.scalar.dma_start(out=dv[:, half:2 * half],
                                    in_=fl[:, half:2 * half])

            def s_col(c):
                off = OFFS[c]
                nc.scalar.activation(
                    out=s_all[:, off:off + CHUNKS[c]], in_=t32s[c],
                    func=mybir.ActivationFunctionType.Copy,
                    bias=1.0, scale=-1.0,
                )
                del t32s[c]

            for c in range(pre):
                load(c)

            for k in range(nch + 3):
                # ACT: smalls first so they never queue behind the square
                if 0 <= k - 2 < nch:
                    u_of(k - 2)
                if 0 <= k - 3 < nch:
                    s_col(k - 3)
                if k < nch:
                    if k + pre < nch:
                        load(k + pre)
                    square(k)
                # DVE: tail (mul+store) first so the store DMA issues early
                # in the iteration; folds of the next chunk follow.
                if 0 <= k - 2 < nch:
                    tail(k - 2)
                if 0 <= k - 1 < nch:
                    folds(k - 1)

            nc.sync.dma_start(out=s.ap(), in_=s_all)

    nc.compile()
    return nc


def host_pack(x16_flat: np.ndarray) -> np.ndarray:
    """[N_CORES*R, D] bf16 row-major -> [N_CORES, P*K*D] chunked d-major."""
    v = x16_flat.view(np.uint16).reshape(N_CORES, P, K, D)
    out = np.empty((N_CORES, P * K * D), dtype=np.uint16)
    pos = 0
    for off, blk in zip(OFFS, CHUNKS):
        blkw = blk * D * P
        # [cores, P, blk, D] -> [cores, P, D, blk]
        b = v[:, :, off:off + blk, :].transpose(0, 1, 3, 2)
        out[:, pos:pos + blkw] = b.reshape(N_CORES, blkw)
        pos += blkw
    return out.view(NP_BF16)


def host_unpack_tx(txd: np.ndarray) -> np.ndarray:
    """[P*K*D] chunked d-major -> [R, D] f32."""
    v = txd.view(np.uint16)
    out = np.empty((P, K, D), dtype=np.uint16)
    pos = 0
    for off, blk in zip(OFFS, CHUNKS):
        blkw = blk * D * P
        b = v[pos:pos + blkw].reshape(P, D, blk)
        out[:, off:off + blk, :] = b.transpose(0, 2, 1)
        pos += blkw
    return out.reshape(R, D).view(NP_BF16).astype(np.float32)


_nc_cache: dict = {}


def _get_nc():
    if "nc" not in _nc_cache:
        _nc_cache["nc"] = build_nc()
    return _nc_cache["nc"]


def kernel(x) -> np.ndarray:
    x = np.asarray(x)
    assert x.shape == (N, D), x.shape
    x16 = np.ascontiguousarray(x.astype(NP_BF16))
    packed = host_pack(x16)
    nc = _get_nc()
    in_maps = [{"x": packed[c]} for c in range(N_CORES)]
    res = run_bass_kernel_spmd(nc, in_maps, core_ids=list(range(N_CORES)))
    out = np.empty((N, D + 1), dtype=np.float32)
    for c, r in enumerate(res.results):
        out[c * R:(c + 1) * R, :D] = host_unpack_tx(r["tx"])
        out[c * R:(c + 1) * R, D] = r["s"].reshape(R).astype(np.float32)
    return out


# revision 19
# speedup vs baseline: 1.0306x; 1.0306x over previous
"""Trainium2 Bass kernel for nn_Projection: out = [(1-s)*x, s],
s = -(1-||x||^2)/(1+||x||^2) per row.

Identity used: with sq = sum(x^2), s = (sq-1)/(sq+1) = 1 - 2/(1+sq).
Let t = 2/(1+sq). Then out = [t*x, 1-t].

HBM-bandwidth bound (elementwise over 512MB in / 516MB out). Gate is
rel_err < 2e-2, so all HBM traffic is bf16 (measured max rel err
~1.2e-2): the host rounds x to bf16, the device computes and stores
bf16, the host upcasts to f32. Halves HBM traffic vs f32.

Layout trick: tiles are d-major in SBUF ([P, D, blk], host pre/post
transposes the per-tile element order). This makes every hot DVE op
eligible for the 2-byte packed 2x perf mode (innermost step 1 on all
operands, including the per-row t broadcast which is stride-0 only
in the middle dim):
  - row-sum of x^2: 7-level fp16 fold tree over d, done IN-PLACE in
    the xsq tile (the write pointer trails both read pointers)
  - t*x multiply: one 2x TT with t16 broadcast over d
GpSimd stays idle: any GpSimd op would serialize with DVE 2x ops on
the exclusive shared SBUF port pair.

Software-pipelined with a 2-chunk skew so the in-order ACT/DVE
queues never stall on each other; loads/stores alternate between the
two HWDGE rings so single-direction phases can use both. The chunk
schedule is variable-size: small chunks at the ends shorten pipeline
fill/drain (dep-chain length scales with chunk size), 64-row chunks
in the middle amortize per-instruction overhead.

Sharding: pure data parallel over rows across 8 NeuronCores.
Per-core row map: row = p*K + (chunk offset + j).
"""

import sys

for _p in ("/opt/trn_rl_repo", "/opt/trn_rl_repo/concourse"):
    if _p not in sys.path:
        sys.path.insert(0, _p)

import ml_dtypes
import numpy as np

import concourse.bacc as bacc
import concourse.tile as tile
from concourse import mybir
from concourse.bass_utils import run_bass_kernel_spmd

N, D = 1048576, 128
N_CORES = 8
R = N // N_CORES   # 131072 rows per core
P = 128            # SBUF partitions
K = R // P         # 1024 rows per partition
BMAX = 64
CHUNKS = [16, 48] + [64] * 14 + [32, 16, 16]  # rows per partition per chunk
assert sum(CHUNKS) == K
OFFS = [sum(CHUNKS[:i]) for i in range(len(CHUNKS))]
BF16 = mybir.dt.bfloat16
FP16 = mybir.dt.float16
F32 = mybir.dt.float32
NP_BF16 = np.dtype(ml_dtypes.bfloat16)


def build_nc(pre: int = 3):
    nch = len(CHUNKS)
    nc = bacc.Bacc(trn_type="TRN2")
    x = nc.dram_tensor("x", [P * K * D], BF16, kind="ExternalInput")
    tx = nc.dram_tensor("tx", [P * K * D], BF16, kind="ExternalOutput")
    s = nc.dram_tensor("s", [P, K], BF16, kind="ExternalOutput")

    def dview(t, c):
        base = OFFS[c] * P * D
        return t.ap()[base:base + P * D * CHUNKS[c]].rearrange(
            "(p f) -> p f", p=P)

    with tile.TileContext(nc) as tc:
        with (
            tc.tile_pool(name="xin", bufs=pre + 6) as x_pool,
            tc.tile_pool(name="sqp", bufs=2) as sq_pool,
            tc.tile_pool(name="small", bufs=4) as small_pool,
            tc.tile_pool(name="singles", bufs=1) as singles,
        ):
            s_all = singles.tile([P, K], BF16)

            x_t = {}     # c -> x tile view [P, D, blk]
            x_fl = {}    # c -> flat x tile
            xsq = {}     # c -> squared tile view
            sqs = {}     # c -> row-sum (fp16 [P, blk])
            us = {}      # c -> u = sq + 0.5 (f32)
            t32s = {}    # c -> 1/u (f32)

            def view(flat, blk):
                return flat[:, 0:D * blk].rearrange("p (d j) -> p d j", j=blk)

            def load(c):
                # Half-transfers on BOTH rings: per-ring throughput caps at
                # ~215-225 GB/s; both together sustain 430+ (direction-
                # agnostic), so every transfer is split across the two.
                blk = CHUNKS[c]
                fl = x_pool.tile([P, D * BMAX], BF16, tag="x", name="x_t")
                half = D * blk // 2
                dv = dview(x, c)
                nc.scalar.dma_start(out=fl[:, 0:half], in_=dv[:, 0:half])
                nc.sync.dma_start(out=fl[:, half:2 * half],
                                  in_=dv[:, half:2 * half])
                x_t[c] = view(fl, blk)
                x_fl[c] = fl

            def square(c):
                fl = sq_pool.tile([P, D * BMAX], FP16, tag="xsq", name="xsq")
                xsq[c] = view(fl, CHUNKS[c])
                nc.scalar.activation(
                    out=xsq[c], in_=x_t[c],
                    func=mybir.ActivationFunctionType.Square,
                    scale=0.7071067811865476,
                )

            def folds(c):
                # In-place fold tree over d: write trails both reads.
                blk = CHUNKS[c]
                a = xsq[c]
                d = D
                while d > 2:
                    d //= 2
                    nc.vector.tensor_add(
                        a[:, 0:d, :], a[:, 0:d, :], a[:, d:2 * d, :])
                sq_f = small_pool.tile([P, BMAX], FP16, tag="sq", name="sq")
                sqs[c] = sq_f[:, 0:blk]
                nc.vector.tensor_add(
                    sqs[c].unsqueeze(1), a[:, 0:1, :], a[:, 1:2, :])
                del xsq[c]

            def u_of(c):
                u_f = small_pool.tile([P, BMAX], F32, tag="u", name="u")
                us[c] = u_f[:, 0:CHUNKS[c]]
                nc.scalar.activation(
                    out=us[c], in_=sqs[c],
                    func=mybir.ActivationFunctionType.Copy, bias=0.5,
                )
                del sqs[c]

            def tail(c):
                blk = CHUNKS[c]
                t_f = small_pool.tile([P, BMAX], F32, tag="t32", name="t32")
                t32s[c] = t_f[:, 0:blk]
                nc.vector.reciprocal_approx_fast(out=t32s[c], in_=us[c])
                del us[c]
                t16f = small_pool.tile([P, BMAX], BF16, tag="t16", name="t16")
                t16 = t16f[:, 0:blk]
                nc.vector.tensor_copy(t16, t32s[c])
                # In-place multiply; 2x mode. (Writing t16 into the x tile
                # would unlock 4x, but 4x on DMA-hot tiles starves the DMA
                # at the SBUF banks - measured 20% slower overall.)
                nc.vector.tensor_mul(
                    x_t[c], x_t[c],
                    t16.unsqueeze(1).broadcast_to([P, D, blk]))
                fl = x_fl[c]
                del x_t[c], x_fl[c]
                half = D * blk // 2
                dv = dview(tx, c)
                nc.sync.dma_start(out=dv[:, 0:half], in_=fl[:, 0:half])
                nc.scalar.dma_start(out=dv[:, half:2 * half],
                                    in_=fl[:, half:2 * half])

            def s_col(c):
                off = OFFS[c]
                nc.scalar.activation(
                    out=s_all[:, off:off + CHUNKS[c]], in_=t32s[c],
                    func=mybir.ActivationFunctionType.Copy,
                    bias=1.0, scale=-1.0,
                )
                del t32s[c]

            for c in range(pre):
                load(c)

            for k in range(nch + 3):
                # ACT: smalls first so they never queue behind the square
                if 0 <= k - 2 < nch:
                    u_of(k - 2)
                if 0 <= k - 3 < nch:
                    s_col(k - 3)
                if k < nch:
                    if k + pre < nch:
                        load(k + pre)
                    square(k)
                # DVE: tail (mul+store) first so the store DMA issues early
                # in the iteration; folds of the next chunk follow.
                if 0 <= k - 2 < nch:
                    tail(k - 2)
                if 0 <= k - 1 < nch:
                    folds(k - 1)

            nc.sync.dma_start(out=s.ap(), in_=s_all)

    nc.compile()
    return nc


def host_pack(x16_flat: np.ndarray) -> np.ndarray:
    """[N_CORES*R, D] bf16 row-major -> [N_CORES, P*K*D] chunked d-major."""
    v = x16_flat.view(np.uint16).reshape(N_CORES, P, K, D)
    out = np.empty((N_CORES, P * K * D), dtype=np.uint16)
    pos = 0
    for off, blk in zip(OFFS, CHUNKS):
        blkw = blk * D * P
        # [cores, P, blk, D] -> [cores, P, D, blk]
        b = v[:, :, off:off + blk, :].transpose(0, 1, 3, 2)
        out[:, pos:pos + blkw] = b.reshape(N_CORES, blkw)
        pos += blkw
    return out.view(NP_BF16)


def host_unpack_tx(txd: np.ndarray) -> np.ndarray:
    """[P*K*D] chunked d-major -> [R, D] f32."""
    v = txd.view(np.uint16)
    out = np.empty((P, K, D), dtype=np.uint16)
    pos = 0
    for off, blk in zip(OFFS, CHUNKS):
        blkw = blk * D * P
        b = v[pos:pos + blkw].reshape(P, D, blk)
        out[:, off:off + blk, :] = b.transpose(0, 2, 1)
        pos += blkw
    return out.reshape(R, D).view(NP_BF16).astype(np.float32)


_nc_cache: dict = {}


def _get_nc():
    if "nc" not in _nc_cache:
        _nc_cache["nc"] = build_nc()
    return _nc_cache["nc"]


def kernel(x) -> np.ndarray:
    x = np.asarray(x)
    assert x.shape == (N, D), x.shape
    x16 = np.ascontiguousarray(x.astype(NP_BF16))
    packed = host_pack(x16)
    nc = _get_nc()
    in_maps = [{"x": packed[c]} for c in range(N_CORES)]
    res = run_bass_kernel_spmd(nc, in_maps, core_ids=list(range(N_CORES)))
    out = np.empty((N, D + 1), dtype=np.float32)
    for c, r in enumerate(res.results):
        out[c * R:(c + 1) * R, :D] = host_unpack_tx(r["tx"])
        out[c * R:(c + 1) * R, D] = r["s"].reshape(R).astype(np.float32)
    return out
